# revision 1
# baseline (speedup 1.0000x reference)
"""Trainium2 Bass kernel for ClipPairWiseLossAll.

loss = sum_{i<j} || relu(r_i - r_j) ||_2   with r = repr[GT], M=512, N=768.

Strategy (8 NeuronCores, SPMD, one shared NEFF):
  * Host: gather r = repr[GT], transpose -> rT [N=768, M=512], cast bf16.
  * Pair space decomposed by DIAGONALS: diagonal o covers pairs (t, t+o),
    t in [0, 512-o). Core c owns o in {16k + (c+1), 16k + (16-c)}, k<32 —
    511 real diagonals + 1 masked dummy, ~16.4k pairs per core.
  * The per-core shift lives in the DATA, not the program: core c receives
    rtab = [rT shifted left by c+1, rT shifted left by 16-c] so the device
    always slices at offset 16k (uniform across cores -> single NEFF).
  * Per k (two diagonals of rounded length L = 512-16k, all 6 n-chunks and
    both slots in single instructions):
      d  = rt2[., t] - rtab[., 16k+t]   one tensor_tensor sub (bf16 2x)
      E  = relu(d)                      one tensor_scalar max-imm (bf16 4x)
      E2 = E^2 -> fp8                   one ACT Square
      psum[row m] += sum_n E2           fp8 DoubleRow one-hot matmuls
  * A per-core mask kills rounded-up columns, ACT computes sqrt with a
    fused row-sum, host adds the 8x64 partials.
"""

import numpy as np

M = 512
N = 768
P = 128
NCH = N // P  # 6
NCORES = 8
NS = 64  # diagonals per core (2 per k)


def _o_list(c):
    out = []
    for k in range(32):
        out.append(16 * k + c + 1)
        out.append(16 * k + 16 - c)
    return out


_PROG = {}

# square-pass engine per k: "act" or "dve" (dve -> bf16 e2, bf16 matmuls)
SQ_DVE_KS = (0,)


def _build_program():
    if "nc" in _PROG:
        return _PROG["nc"]

    from contextlib import ExitStack

    import concourse.bass as bass
    import concourse.bacc as bacc
    import concourse.tile as tile
    from concourse import mybir

    AOT = mybir.AluOpType
    AFT = mybir.ActivationFunctionType
    bf16 = mybir.dt.bfloat16
    fp8 = mybir.dt.float8e4
    f32 = mybir.dt.float32

    nc = bacc.Bacc(
        "TRN2",
        target_bir_lowering=False,
        debug=False,
        enable_asserts=False,
        num_devices=NCORES,
    )

    rt_d = nc.dram_tensor("rt", [P, NCH * M], bf16, kind="ExternalInput")
    rtab_d = nc.dram_tensor("rtab", [P, 2 * NCH * M], bf16, kind="ExternalInput")
    oh_d = nc.dram_tensor("oh", [P, NS * 2 * NS], fp8, kind="ExternalInput")
    out_d = nc.dram_tensor("out", [NS, 1], f32, kind="ExternalOutput")

    with ExitStack() as ctx:
        tc = ctx.enter_context(tile.TileContext(nc))
        singles = ctx.enter_context(tc.tile_pool(name="singles", bufs=1))
        dpool = ctx.enter_context(tc.tile_pool(name="d", bufs=4))
        epool = ctx.enter_context(tc.tile_pool(name="e", bufs=4))
        e2pool = ctx.enter_context(tc.tile_pool(name="e2", bufs=4))
        pspool = ctx.enter_context(tc.tile_pool(name="ps", bufs=1, space="PSUM"))

        # one-hot lhsT stack first (PE needs it for the very first matmul),
        # on the GPSIMD SWDGE queue so it runs parallel to the sync-queue DMAs
        oh = singles.tile([P, NS, 2, NS], fp8)
        nc.gpsimd.dma_start(out=oh, in_=oh_d.ap())
        # piecewise rt/rtab DMAs, ordered so the first (smallest-L) compute
        # iterations can start as soon as their slices arrive
        NPC = 4
        PCM = M // NPC
        rt_sb = singles.tile([P, NCH, M], bf16)
        rt_view = rt_d.ap().rearrange("p (c t) -> p c t", c=NCH)
        rtab_sb = singles.tile([P, 2, NCH, M], bf16)
        rtab_view = rtab_d.ap().rearrange("p (s c t) -> p s c t", s=2, c=NCH)
        for pc in range(NPC):
            lo, hi = pc * PCM, (pc + 1) * PCM
            nc.sync.dma_start(out=rt_sb[:, :, lo:hi], in_=rt_view[:, :, lo:hi])
            lo2, hi2 = M - hi, M - lo
            nc.sync.dma_start(
                out=rtab_sb[:, :, :, lo2:hi2], in_=rtab_view[:, :, :, lo2:hi2]
            )

        ps = pspool.tile([NS, M], f32)
        nc.vector.memset(ps, 0.0)

        # bf16 one-hot lhsT rows for the DVE-squared k's
        ohb = singles.tile([P, 2 * len(SQ_DVE_KS), NS], bf16)
        nc.vector.memset(ohb, 0.0)
        _ohb_col = {}
        for j, kq in enumerate(SQ_DVE_KS):
            for slot in range(2):
                m = 2 * kq + slot
                jj = 2 * j + slot
                _ohb_col[m] = jj
                nc.vector.memset(ohb[:, jj, m : m + 1], 1.0)

        for k in range(31, -1, -1):
            L = M - 16 * k
            d_t = dpool.tile([P, 2, NCH, M], bf16, tag="d")
            in0s = rt_sb[:, :, 0:L]
            in0 = bass.AP(
                tensor=in0s.tensor,
                offset=in0s.offset,
                ap=[in0s.ap[0], [0, 2], in0s.ap[1], in0s.ap[2]],
            )
            nc.vector.tensor_sub(
                d_t[:, :, :, 0:L],
                in0,
                rtab_sb[:, :, :, 16 * k : 16 * k + L],
            )
            e_t = epool.tile([P, 2, NCH, M], bf16, tag="e")
            nc.vector.tensor_scalar(
                out=e_t[:, :, :, 0:L],
                in0=d_t[:, :, :, 0:L],
                scalar1=0.0,
                scalar2=None,
                op0=AOT.max,
            )
            if k in SQ_DVE_KS:
                e2b_t = e2pool.tile([P, 2, NCH, M], bf16, tag="e2b")
                nc.vector.tensor_mul(
                    e2b_t[:, :, :, 0:L], e_t[:, :, :, 0:L], e_t[:, :, :, 0:L]
                )
                for slot in range(2):
                    m = 2 * k + slot
                    for c in range(NCH):
                        nc.tensor.matmul(
                            ps[:, 0:L],
                            ohb[:, _ohb_col[m], :],
                            e2b_t[:, slot, c, 0:L],
                            start=False,
                            stop=False,
                            skip_group_check=True,
                        )
            else:
                e2_t = e2pool.tile([P, 2, NCH, M], fp8, tag="e2")
                nc.scalar.activation(
                    out=e2_t[:, :, :, 0:L],
                    in_=e_t[:, :, :, 0:L],
                    func=AFT.Square,
                )
                for slot in range(2):
                    m = 2 * k + slot
                    for c2 in range(NCH // 2):
                        nc.tensor.matmul(
                            ps[:, 0:L],
                            oh[:, m, :, :],
                            e2_t[:, slot, 2 * c2 : 2 * c2 + 2, 0:L],
                            start=False,
                            stop=False,
                            skip_group_check=True,
                            perf_mode=mybir.MatmulPerfMode.DoubleRow,
                        )

        sqrt_t = singles.tile([NS, M], bf16)
        res = singles.tile([NS, 1], f32)
        nc.scalar.activation(out=sqrt_t, in_=ps[:, :], func=AFT.Sqrt, accum_out=res)
        nc.sync.dma_start(out=out_d.ap(), in_=res)

    nc.compile()
    _PROG["nc"] = nc
    return nc


def _shift_pc(rT_bf, h):
    """rT shifted left by h columns, HUGE-padded, in [p, chunk, t] layout.

    The pad makes relu(r_t - pad) exactly 0, so rounded-up columns
    contribute nothing and no mask pass is needed."""
    N_, M_ = rT_bf.shape
    sh = np.full_like(rT_bf, 3.0e38)
    if h < M_:
        sh[:, : M_ - h] = rT_bf[:, h:]
    return np.transpose(sh.reshape(NCH, P, M_), (1, 0, 2))  # [P, NCH, M]


def _in_maps(repr_np, GT_np):
    import ml_dtypes

    r = np.asarray(repr_np, dtype=np.float32)[np.asarray(GT_np).astype(np.int64)]
    rT = np.ascontiguousarray(r.T)  # [N, M] f32
    rT_bf = rT.astype(ml_dtypes.bfloat16)

    base = np.transpose(rT_bf.reshape(NCH, P, M), (1, 0, 2))  # [P, NCH, M]
    rt = np.ascontiguousarray(base).reshape(P, -1)

    ohs = np.zeros((P, NS, 2, NS), dtype=ml_dtypes.float8_e4m3)
    for m in range(NS):
        ohs[:, m, :, m] = 1.0
    ohs = ohs.reshape(P, NS * 2 * NS)

    maps = []
    for c in range(NCORES):
        rtab = np.stack(
            [_shift_pc(rT_bf, c + 1), _shift_pc(rT_bf, 16 - c)], axis=1
        ).reshape(P, -1)
        maps.append({"rt": rt, "rtab": np.ascontiguousarray(rtab), "oh": ohs})
    return maps


def run_device(repr_np, GT_np, trace=False, trace_cores=None):
    """Run the bass kernel on 8 cores; returns (total, BassKernelResults)."""
    from concourse.bass_utils import run_bass_kernel_spmd

    nc = _build_program()
    maps = _in_maps(repr_np, GT_np)
    res = run_bass_kernel_spmd(
        nc,
        maps,
        core_ids=list(range(NCORES)),
        trace=trace,
        trace_cores=trace_cores,
    )
    total = 0.0
    for core_out in res.results:
        total += float(core_out["out"].astype(np.float64).sum())
    return np.float32(total), res


def kernel(repr, GT):
    total, _ = run_device(repr, GT, trace=False)
    return total



# revision 5
# speedup vs baseline: 1.1562x; 1.1562x over previous
"""Trainium2 Bass kernel for ClipPairWiseLossAll.

loss = sum_{i<j} || relu(r_i - r_j) ||_2   with r = repr[GT], M=512, N=768.

Pair space is split into two exactly-composing parts (8 cores, SPMD,
one shared NEFF; per-core behavior lives in the DMA'd data):

COLUMN part (j < 192; 60% of pairs, long streams):
  For a fixed j, e[n, i] = max(rT[n, i] - rT[n, j], 0) over i in
  [16m, 512), m = j//16. The subtrahend is a per-partition scalar, so a
  single 4x tensor_scalar (op0=subtract with f32 scalar AP, op1=max 0)
  computes sub+relu in one pass. Core c owns j in {16m+c, 16m+15-c},
  m < 12 -> 24 "A-slots". The i <= j sub-range is computed (relu
  garbage) and killed by a final mask.

DIAGONAL part (pairs (t, t+o), o < 320, t >= 192; short streams):
  Baseline-style: core c owns o in {16k+c+1, 16k+16-c}, k < 20 -> 40
  "D-slots". The per-core shift lives in rtab (rT shifted left by
  192+delta, HUGE-padded so rounded-up tails relu to exactly 0), so the
  device slices at uniform offset 16k. One 2x tensor_tensor sub + one
  4x tensor_scalar relu per k covers both slots and all 6 chunks.

Squares: ACT Square -> fp8 (PE DoubleRow reduce) for most groups; DVE
tensor_mul -> bf16 (plain PE reduce) for a few small groups to balance
DVE vs ACT. One-hot lhsT matrices are synthesized on-device from a
single [:, 64] = 1 column via shifted views. ps [64, 512] f32 x2 banks
(step parity); tail = add banks, mask, ACT Sqrt with fused row-sum;
host sums the 8x64 partials.
"""

import numpy as np

M = 512
N = 768
P = 128
NCH = N // P  # 6
NCORES = 8
NS = 64
NMA = 12  # column groups (A), m = 0..11, slots 0..23
NKD = 20  # diagonal groups (D), k = 0..19, slots 24..63
TMIN = 192  # diagonal part covers t >= TMIN (j >= 192)

# groups whose square runs on DVE (bf16 matmuls) instead of ACT (fp8 DR)
SQ_DVE_STEPS = frozenset({("D", 18), ("D", 19)})

HUGE = 3.0e38

_PROG = {}


def _emit_order():
    """Interleave D (k desc) and A (m desc) round-robin, 20:12."""
    order = []
    dk = list(range(NKD - 1, -1, -1))
    am = list(range(NMA - 1, -1, -1))
    while dk or am:
        if dk:
            order.append(("D", dk.pop(0)))
        if dk and (len(dk) % 2 == 0):
            order.append(("D", dk.pop(0)))
        if am:
            order.append(("A", am.pop(0)))
    return order


def _build_program():
    if "nc" in _PROG:
        return _PROG["nc"]

    from contextlib import ExitStack

    import concourse.bass as bass
    import concourse.bacc as bacc
    import concourse.tile as tile
    from concourse import mybir

    AOT = mybir.AluOpType
    AFT = mybir.ActivationFunctionType
    bf16 = mybir.dt.bfloat16
    fp8 = mybir.dt.float8e4
    f32 = mybir.dt.float32

    nc = bacc.Bacc(
        "TRN2",
        target_bir_lowering=False,
        debug=False,
        enable_asserts=False,
        num_devices=NCORES,
    )

    LD_MAX = 320
    rt_d = nc.dram_tensor("rt", [P, NCH * M], bf16, kind="ExternalInput")
    rtab_d = nc.dram_tensor("rtab", [P, 2 * NCH * LD_MAX], bf16, kind="ExternalInput")
    bias_d = nc.dram_tensor("bias", [P, NCH * 2 * NMA], f32, kind="ExternalInput")
    mask_d = nc.dram_tensor("mask", [NS, M], f32, kind="ExternalInput")
    out_d = nc.dram_tensor("out", [NS, 1], f32, kind="ExternalOutput")

    with ExitStack() as ctx:
        tc = ctx.enter_context(tile.TileContext(nc))
        singles = ctx.enter_context(tc.tile_pool(name="singles", bufs=1))
        epool = ctx.enter_context(tc.tile_pool(name="e", bufs=4))
        e2pool = ctx.enter_context(tc.tile_pool(name="e2", bufs=4))
        e2bpool = ctx.enter_context(tc.tile_pool(name="e2b", bufs=2))
        pspool = ctx.enter_context(tc.tile_pool(name="ps", bufs=1, space="PSUM"))

        rt_sb = singles.tile([P, NCH, M], bf16)
        rtab_sb = singles.tile([P, 2, NCH, LD_MAX], bf16)
        bias_sb = singles.tile([P, NCH, 2 * NMA], f32)
        mask_sb = singles.tile([NS, M], f32)

        rt_view = rt_d.ap().rearrange("p (c t) -> p c t", c=NCH)
        rtab_view = rtab_d.ap().rearrange("p (s c t) -> p s c t", s=2, c=NCH)

        # D runs first (k desc): rt cols desc from 512; rtab cols desc from 320
        nc.gpsimd.dma_start(
            out=bias_sb, in_=bias_d.ap().rearrange("p (c s) -> p c s", c=NCH)
        )
        nc.gpsimd.dma_start(out=mask_sb, in_=mask_d.ap())
        # first piece [176:512] serves every D-group and A11; rest descend
        lo_prev = M
        for cut in [176, 96, 0]:
            nc.sync.dma_start(
                out=rt_sb[:, :, cut:lo_prev], in_=rt_view[:, :, cut:lo_prev]
            )
            lo_prev = cut
        lo_prev = LD_MAX
        for cut in [288, 224, 112, 0]:
            nc.gpsimd.dma_start(
                out=rtab_sb[:, :, :, cut:lo_prev], in_=rtab_view[:, :, :, cut:lo_prev]
            )
            lo_prev = cut

        # one-hot lhsT banks: col s of view [:, 64-s : 128-s] is 1
        ohb = singles.tile([P, 2 * NS], bf16)
        nc.vector.memset(ohb, 0.0)
        nc.vector.memset(ohb[:, NS : NS + 1], 1.0)
        oh8 = singles.tile([P, 2, 2 * NS], fp8)
        nc.vector.memset(oh8, 0.0)
        nc.vector.memset(oh8[:, :, NS : NS + 1], 1.0)

        ps0 = pspool.tile([NS, M], f32)
        ps1 = pspool.tile([NS, M], f32)
        nc.vector.memset(ps0, 0.0)
        nc.vector.memset(ps1, 0.0)
        pss = [ps0, ps1]

        for step_i, (kind, idx) in enumerate(_emit_order()):
            ps = pss[step_i % 2]
            e_t = epool.tile([P, 2, NCH, M], bf16, tag="e")

            if kind == "A":
                m = idx
                L = M - 16 * m
                s_base = 2 * m
                for sl in range(2):
                    s = s_base + sl
                    for c in range(NCH):
                        nc.vector.tensor_scalar(
                            out=e_t[:, sl, c, 0:L],
                            in0=rt_sb[:, c, 16 * m : M],
                            scalar1=bias_sb[:, c, s : s + 1],
                            scalar2=0.0,
                            op0=AOT.subtract,
                            op1=AOT.max,
                        )
            else:
                k = idx
                L = LD_MAX - 16 * k
                s_base = 2 * NMA + 2 * k
                in0s = rt_sb[:, :, TMIN : TMIN + L]
                in0 = bass.AP(
                    tensor=in0s.tensor,
                    offset=in0s.offset,
                    ap=[in0s.ap[0], [0, 2], in0s.ap[1], in0s.ap[2]],
                )
                nc.vector.tensor_sub(
                    e_t[:, :, :, 0:L],
                    in0,
                    rtab_sb[:, :, :, 16 * k : 16 * k + L],
                )
                nc.vector.tensor_scalar(
                    out=e_t[:, :, :, 0:L],
                    in0=e_t[:, :, :, 0:L],
                    scalar1=0.0,
                    scalar2=None,
                    op0=AOT.max,
                )

            if (kind, idx) in SQ_DVE_STEPS:
                e2b = e2bpool.tile([P, 2, NCH, M], bf16, tag="e2b")
                nc.vector.tensor_mul(
                    e2b[:, :, :, 0:L], e_t[:, :, :, 0:L], e_t[:, :, :, 0:L]
                )
                for sl in range(2):
                    s = s_base + sl
                    for c in range(NCH):
                        nc.tensor.matmul(
                            ps[:, 0:L],
                            ohb[:, NS - s : 2 * NS - s],
                            e2b[:, sl, c, 0:L],
                            start=False,
                            stop=False,
                            skip_group_check=True,
                        )
            else:
                e2 = e2pool.tile([P, 2, NCH, M], fp8, tag="e2")
                nc.scalar.activation(
                    out=e2[:, :, :, 0:L], in_=e_t[:, :, :, 0:L], func=AFT.Square
                )
                for sl in range(2):
                    s = s_base + sl
                    for c2 in range(NCH // 2):
                        nc.tensor.matmul(
                            ps[:, 0:L],
                            oh8[:, :, NS - s : 2 * NS - s],
                            e2[:, sl, 2 * c2 : 2 * c2 + 2, 0:L],
                            start=False,
                            stop=False,
                            skip_group_check=True,
                            perf_mode=mybir.MatmulPerfMode.DoubleRow,
                        )

        t0 = singles.tile([NS, M], f32)
        nc.vector.tensor_mul(t0, ps0, mask_sb)
        t1 = singles.tile([NS, M], f32)
        nc.vector.tensor_mul(t1, ps1, mask_sb)
        s2 = singles.tile([NS, M], f32)
        nc.vector.tensor_add(s2, t0, t1)
        sq = singles.tile([NS, M], bf16)
        res = singles.tile([NS, 1], f32)
        nc.scalar.activation(out=sq, in_=s2, func=AFT.Sqrt, accum_out=res)
        nc.sync.dma_start(out=out_d.ap(), in_=res)

    nc.compile()
    _PROG["nc"] = nc
    return nc


def _in_maps(repr_np, GT_np):
    import ml_dtypes

    LD_MAX = 320
    r = np.asarray(repr_np, dtype=np.float32)[np.asarray(GT_np).astype(np.int64)]
    rT = np.ascontiguousarray(r.T)  # [N, M] f32
    rT_bf = rT.astype(ml_dtypes.bfloat16)

    # rt[p, c, i] = rT[c*128+p, i]
    rt = np.ascontiguousarray(
        np.transpose(rT_bf.reshape(NCH, P, M), (1, 0, 2))
    ).reshape(P, -1)
    rT_f32_p = np.transpose(rT.reshape(NCH, P, M), (1, 0, 2))  # [P, NCH, M] f32

    maps = []
    for c in range(NCORES):
        # bias[p, ch, s] = rT[ch*128+p, j_s] for A-slots s = 2m+sl
        js_a = []
        for m in range(NMA):
            js_a += [16 * m + c, 16 * m + 15 - c]
        bias = np.ascontiguousarray(rT_f32_p[:, :, js_a].astype(np.float32)).reshape(
            P, -1
        )

        # rtab[p, sl, ch, u] = rT[ch*128+p, TMIN + delta_sl + u], HUGE-padded
        rtab = np.full((2, N, LD_MAX), HUGE, dtype=np.float32)
        for sl, delta in enumerate((c + 1, 16 - c)):
            shift = TMIN + delta
            avail = M - shift
            take = min(avail, LD_MAX)
            rtab[sl, :, :take] = rT[:, shift : shift + take]
        rtab_bf = rtab.astype(ml_dtypes.bfloat16)
        rtab_p = np.transpose(rtab_bf.reshape(2, NCH, P, LD_MAX), (2, 0, 1, 3))
        rtab_p = np.ascontiguousarray(rtab_p).reshape(P, -1)

        # mask[s, u]: A-slots kill i <= j (u <= j - 16m); D-slots all-pass
        mask = np.zeros((NS, M), dtype=np.float32)
        for m in range(NMA):
            L = M - 16 * m
            for sl, j in enumerate((16 * m + c, 16 * m + 15 - c)):
                u_lo = j - 16 * m + 1
                mask[2 * m + sl, u_lo:L] = 1.0
        for k in range(NKD):
            L = LD_MAX - 16 * k
            mask[2 * NMA + 2 * k, 0:L] = 1.0
            mask[2 * NMA + 2 * k + 1, 0:L] = 1.0

        maps.append({"rt": rt, "rtab": rtab_p, "bias": bias, "mask": mask})
    return maps


def run_device(repr_np, GT_np, trace=False, trace_cores=None):
    """Run the bass kernel on 8 cores; returns (total, BassKernelResults)."""
    from concourse.bass_utils import run_bass_kernel_spmd

    nc = _build_program()
    maps = _in_maps(repr_np, GT_np)
    res = run_bass_kernel_spmd(
        nc,
        maps,
        core_ids=list(range(NCORES)),
        trace=trace,
        trace_cores=trace_cores,
    )
    total = 0.0
    for core_out in res.results:
        total += float(core_out["out"].astype(np.float64).sum())
    return np.float32(total), res


def kernel(repr, GT):
    total, _ = run_device(repr, GT, trace=False)
    return total


# revision 9
# speedup vs baseline: 1.1596x; 1.0029x over previous
"""Trainium2 Bass kernel for ClipPairWiseLossAll.

loss = sum_{i<j} || relu(r_i - r_j) ||_2   with r = repr[GT], M=512, N=768.

Pair space is split into two exactly-composing parts (8 cores, SPMD,
one shared NEFF; per-core behavior lives in the DMA'd data):

COLUMN part (j < 192; 60% of pairs, long streams):
  For a fixed j, e[n, i] = max(rT[n, i] - rT[n, j], 0) over i in
  [16m, 512), m = j//16. The subtrahend is a per-partition scalar, so a
  single 4x tensor_scalar (op0=subtract with f32 scalar AP, op1=max 0)
  computes sub+relu in one pass. Core c owns j in {16m+c, 16m+15-c},
  m < 12 -> 24 "A-slots". The i <= j sub-range is computed (relu
  garbage) and killed by a final mask.

DIAGONAL part (pairs (t, t+o), o < 320, t >= 192; short streams):
  Baseline-style: core c owns o in {16k+c+1, 16k+16-c}, k < 20 -> 40
  "D-slots". The per-core shift lives in rtab (rT shifted left by
  192+delta, HUGE-padded so rounded-up tails relu to exactly 0), so the
  device slices at uniform offset 16k. One 2x tensor_tensor sub + one
  4x tensor_scalar relu per k covers both slots and all 6 chunks.

Squares: ACT Square -> fp8 for most groups, DVE tensor_mul -> fp8 (1x)
for a few small groups to balance DVE vs ACT; every group reduces with
fp8 DoubleRow one-hot matmuls. Slots are split by step parity into two
ps [32, 512] f32 banks (32-row one-hots halve LDWEIGHTS); one-hots are
synthesized on-device from a single [:, 32] = 1 column via shifted
views. Tail per bank: mask multiply, ACT Sqrt with fused row-sum; host
adds the 8x64 partials.
"""

import numpy as np

M = 512
N = 768
P = 128
NCH = N // P  # 6
NCORES = 8
NS = 64
NR = 32  # ps rows per parity bank
NMA = 12  # column groups (A), m = 0..11
NKD = 20  # diagonal groups (D), k = 0..19
TMIN = 192  # diagonal part covers t >= TMIN (j >= 192)

# groups whose square runs on DVE (fp8 tensor_mul) instead of ACT
SQ_DVE_STEPS = frozenset({("D", 18), ("D", 19)})

HUGE = 3.0e38

_PROG = {}


def _emit_order():
    """Interleave D (k desc) and A (m desc) round-robin, 20:12."""
    order = []
    dk = list(range(NKD - 1, -1, -1))
    am = list(range(NMA - 1, -1, -1))
    while dk or am:
        if dk:
            order.append(("D", dk.pop(0)))
        if dk and (len(dk) % 2 == 0):
            order.append(("D", dk.pop(0)))
        if am:
            order.append(("A", am.pop(0)))
    return order


def _build_program():
    if "nc" in _PROG:
        return _PROG["nc"]

    from contextlib import ExitStack

    import concourse.bass as bass
    import concourse.bacc as bacc
    import concourse.tile as tile
    from concourse import mybir

    AOT = mybir.AluOpType
    AFT = mybir.ActivationFunctionType
    bf16 = mybir.dt.bfloat16
    fp8 = mybir.dt.float8e4
    f32 = mybir.dt.float32

    nc = bacc.Bacc(
        "TRN2",
        target_bir_lowering=False,
        debug=False,
        enable_asserts=False,
        num_devices=NCORES,
    )

    LD_MAX = 320
    rt_d = nc.dram_tensor("rt", [P, NCH * M], bf16, kind="ExternalInput")
    rtab_d = nc.dram_tensor("rtab", [P, 2 * NCH * LD_MAX], bf16, kind="ExternalInput")
    bias_d = nc.dram_tensor("bias", [P, NCH * 2 * NMA], f32, kind="ExternalInput")
    mask_d = nc.dram_tensor("mask", [NS, M], f32, kind="ExternalInput")
    out_d = nc.dram_tensor("out", [NS, 1], f32, kind="ExternalOutput")

    with ExitStack() as ctx:
        tc = ctx.enter_context(tile.TileContext(nc))
        singles = ctx.enter_context(tc.tile_pool(name="singles", bufs=1))
        epool = ctx.enter_context(tc.tile_pool(name="e", bufs=4))
        e2pool = ctx.enter_context(tc.tile_pool(name="e2", bufs=4))
        pspool = ctx.enter_context(tc.tile_pool(name="ps", bufs=1, space="PSUM"))

        rt_sb = singles.tile([P, NCH, M], bf16)
        rtab_sb = singles.tile([P, 2, NCH, LD_MAX], bf16)
        bias_sb = singles.tile([P, NCH, 2 * NMA], f32)
        mask0 = singles.tile([NR, M], f32)
        mask1 = singles.tile([NR, M], f32)

        rt_view = rt_d.ap().rearrange("p (c t) -> p c t", c=NCH)
        rtab_view = rtab_d.ap().rearrange("p (s c t) -> p s c t", s=2, c=NCH)

        nc.gpsimd.dma_start(
            out=bias_sb, in_=bias_d.ap().rearrange("p (c s) -> p c s", c=NCH)
        )
        nc.gpsimd.dma_start(out=mask0, in_=mask_d.ap()[0:NR, :])
        nc.gpsimd.dma_start(out=mask1, in_=mask_d.ap()[NR:NS, :])
        # first piece [176:512] serves every D-group and A11; rest descend
        lo_prev = M
        for cut in [176, 96, 0]:
            nc.sync.dma_start(
                out=rt_sb[:, :, cut:lo_prev], in_=rt_view[:, :, cut:lo_prev]
            )
            lo_prev = cut
        lo_prev = LD_MAX
        for cut in [288, 224, 112, 0]:
            nc.gpsimd.dma_start(
                out=rtab_sb[:, :, :, cut:lo_prev], in_=rtab_view[:, :, :, cut:lo_prev]
            )
            lo_prev = cut

        # one-hot lhsT bank: col r of view [:, :, 32-r : 64-r] is 1
        oh8 = singles.tile([P, 2, 2 * NR], fp8)
        nc.vector.memset(oh8, 0.0)
        nc.vector.memset(oh8[:, :, NR : NR + 1], 1.0)

        ps0 = pspool.tile([NR, M], f32)
        ps1 = pspool.tile([NR, M], f32)
        nc.vector.memset(ps0, 0.0)
        nc.vector.memset(ps1, 0.0)
        pss = [ps0, ps1]

        for step_i, (kind, idx) in enumerate(_emit_order()):
            ps = pss[step_i % 2]
            r_base = 2 * (step_i // 2)
            e_t = epool.tile([P, 2, NCH, M], bf16, tag="e")

            if kind == "A":
                m = idx
                L = M - 16 * m
                for sl in range(2):
                    s = 2 * m + sl
                    for c in range(NCH):
                        nc.vector.tensor_scalar(
                            out=e_t[:, sl, c, 0:L],
                            in0=rt_sb[:, c, 16 * m : M],
                            scalar1=bias_sb[:, c, s : s + 1],
                            scalar2=0.0,
                            op0=AOT.subtract,
                            op1=AOT.max,
                        )
            else:
                k = idx
                L = LD_MAX - 16 * k
                in0s = rt_sb[:, :, TMIN : TMIN + L]
                in0 = bass.AP(
                    tensor=in0s.tensor,
                    offset=in0s.offset,
                    ap=[in0s.ap[0], [0, 2], in0s.ap[1], in0s.ap[2]],
                )
                nc.vector.tensor_sub(
                    e_t[:, :, :, 0:L],
                    in0,
                    rtab_sb[:, :, :, 16 * k : 16 * k + L],
                )
                nc.vector.tensor_scalar(
                    out=e_t[:, :, :, 0:L],
                    in0=e_t[:, :, :, 0:L],
                    scalar1=0.0,
                    scalar2=None,
                    op0=AOT.max,
                )

            e2 = e2pool.tile([P, 2, NCH, M], fp8, tag="e2")
            if (kind, idx) in SQ_DVE_STEPS:
                nc.vector.tensor_mul(
                    e2[:, :, :, 0:L], e_t[:, :, :, 0:L], e_t[:, :, :, 0:L]
                )
            else:
                nc.scalar.activation(
                    out=e2[:, :, :, 0:L], in_=e_t[:, :, :, 0:L], func=AFT.Square
                )
            for sl in range(2):
                r = r_base + sl
                for c2 in range(NCH // 2):
                    nc.tensor.matmul(
                        ps[:, 0:L],
                        oh8[:, :, NR - r : 2 * NR - r],
                        e2[:, sl, 2 * c2 : 2 * c2 + 2, 0:L],
                        start=False,
                        stop=False,
                        skip_group_check=True,
                        perf_mode=mybir.MatmulPerfMode.DoubleRow,
                    )

        for par, (ps, msk) in enumerate(((ps0, mask0), (ps1, mask1))):
            t_m = singles.tile([NR, M], f32, name=f"t{par}")
            nc.vector.tensor_mul(t_m, ps, msk)
            sq = singles.tile([NR, M], bf16, name=f"sq{par}")
            res = singles.tile([NR, 1], f32, name=f"res{par}")
            nc.scalar.activation(out=sq, in_=t_m, func=AFT.Sqrt, accum_out=res)
            nc.sync.dma_start(out=out_d.ap()[par * NR : (par + 1) * NR, :], in_=res)

    nc.compile()
    _PROG["nc"] = nc
    return nc


def _slot_infos():
    """Device-order slot list: (parity, row, kind, idx, sl)."""
    infos = []
    for step_i, (kind, idx) in enumerate(_emit_order()):
        for sl in range(2):
            infos.append((step_i % 2, 2 * (step_i // 2) + sl, kind, idx, sl))
    return infos


def _in_maps(repr_np, GT_np):
    import ml_dtypes

    LD_MAX = 320
    r = np.asarray(repr_np, dtype=np.float32)[np.asarray(GT_np).astype(np.int64)]
    rT = np.ascontiguousarray(r.T)  # [N, M] f32
    rT_bf = rT.astype(ml_dtypes.bfloat16)

    rt = np.ascontiguousarray(
        np.transpose(rT_bf.reshape(NCH, P, M), (1, 0, 2))
    ).reshape(P, -1)
    rT_f32_p = np.transpose(rT.reshape(NCH, P, M), (1, 0, 2))  # [P, NCH, M] f32

    infos = _slot_infos()
    maps = []
    for c in range(NCORES):
        js_a = []
        for m in range(NMA):
            js_a += [16 * m + c, 16 * m + 15 - c]
        bias = np.ascontiguousarray(rT_f32_p[:, :, js_a].astype(np.float32)).reshape(
            P, -1
        )

        rtab = np.full((2, N, LD_MAX), HUGE, dtype=np.float32)
        for sl, delta in enumerate((c + 1, 16 - c)):
            shift = TMIN + delta
            take = min(M - shift, LD_MAX)
            rtab[sl, :, :take] = rT[:, shift : shift + take]
        rtab_bf = rtab.astype(ml_dtypes.bfloat16)
        rtab_p = np.transpose(rtab_bf.reshape(2, NCH, P, LD_MAX), (2, 0, 1, 3))
        rtab_p = np.ascontiguousarray(rtab_p).reshape(P, -1)

        # mask rows: [parity*NR + row, u]
        mask = np.zeros((NS, M), dtype=np.float32)
        for par, row, kind, idx, sl in infos:
            mrow = par * NR + row
            if kind == "A":
                m = idx
                L = M - 16 * m
                j = 16 * m + c if sl == 0 else 16 * m + 15 - c
                mask[mrow, j - 16 * m + 1 : L] = 1.0
            else:
                k = idx
                L = LD_MAX - 16 * k
                mask[mrow, 0:L] = 1.0

        maps.append({"rt": rt, "rtab": rtab_p, "bias": bias, "mask": mask})
    return maps


def run_device(repr_np, GT_np, trace=False, trace_cores=None):
    """Run the bass kernel on 8 cores; returns (total, BassKernelResults)."""
    from concourse.bass_utils import run_bass_kernel_spmd

    nc = _build_program()
    maps = _in_maps(repr_np, GT_np)
    res = run_bass_kernel_spmd(
        nc,
        maps,
        core_ids=list(range(NCORES)),
        trace=trace,
        trace_cores=trace_cores,
    )
    total = 0.0
    for core_out in res.results:
        total += float(core_out["out"].astype(np.float64).sum())
    return np.float32(total), res


def kernel(repr, GT):
    total, _ = run_device(repr, GT, trace=False)
    return total


# revision 12
# speedup vs baseline: 1.1670x; 1.0064x over previous
"""Trainium2 Bass kernel for ClipPairWiseLossAll.

loss = sum_{i<j} || relu(r_i - r_j) ||_2   with r = repr[GT], M=512, N=768.

Pair space is split into two exactly-composing parts (8 cores, SPMD,
one shared NEFF; per-core behavior lives in the DMA'd data):

COLUMN part (j < 192; 60% of pairs, long streams):
  For a fixed j, e[n, i] = max(rT[n, i] - rT[n, j], 0) over i in
  [16m, 512), m = j//16. The subtrahend is a per-partition scalar, so a
  single 4x tensor_scalar (op0=subtract with f32 scalar AP, op1=max 0)
  computes sub+relu in one pass. Core c owns j in {16m+c, 16m+15-c},
  m < 12 -> 24 "A-slots". The i <= j sub-range is computed (relu
  garbage) and killed by a final mask.

DIAGONAL part (pairs (t, t+o), o < 320, t >= 192; short streams):
  Baseline-style: core c owns o in {16k+c+1, 16k+16-c}, k < 20 -> 40
  "D-slots". The per-core shift lives in rtab (rT shifted left by
  192+delta, HUGE-padded so rounded-up tails relu to exactly 0), so the
  device slices at uniform offset 16k. One 2x tensor_tensor sub + one
  4x tensor_scalar relu per k covers both slots and all 6 chunks.

Squares: ACT Square -> fp8 for most groups, DVE tensor_mul -> fp8 (1x)
for a few small groups to balance DVE vs ACT; every group reduces with
fp8 DoubleRow one-hot matmuls. Slots are split by step parity into two
ps [32, 512] f32 banks (32-row one-hots halve LDWEIGHTS); one-hots are
synthesized on-device from a single [:, 32] = 1 column via shifted
views. Tail per bank: mask multiply, ACT Sqrt with fused row-sum; host
adds the 8x64 partials.
"""

import numpy as np

M = 512
N = 768
P = 128
NCH = N // P  # 6
NCORES = 8
NS = 64
NR = 32  # ps rows per parity bank
NMA = 12  # column groups (A), m = 0..11
NKD = 20  # diagonal groups (D), k = 0..19
TMIN = 192  # diagonal part covers t >= TMIN (j >= 192)

# groups whose square runs on DVE (fp8 tensor_mul) instead of ACT
SQ_DVE_STEPS = frozenset({("D", 18), ("D", 19)})

HUGE = 3.0e38

_PROG = {}


def _emit_order():
    """Interleave D (k desc) and A (m desc) round-robin, 20:12."""
    order = []
    dk = list(range(NKD - 1, -1, -1))
    am = list(range(NMA - 1, -1, -1))
    while dk or am:
        if dk:
            order.append(("D", dk.pop(0)))
        if dk and (len(dk) % 2 == 0):
            order.append(("D", dk.pop(0)))
        if am:
            order.append(("A", am.pop(0)))
    return order


def _build_program():
    if "nc" in _PROG:
        return _PROG["nc"]

    from contextlib import ExitStack

    import concourse.bass as bass
    import concourse.bacc as bacc
    import concourse.tile as tile
    from concourse import mybir

    AOT = mybir.AluOpType
    AFT = mybir.ActivationFunctionType
    bf16 = mybir.dt.bfloat16
    fp8 = mybir.dt.float8e4
    f32 = mybir.dt.float32

    nc = bacc.Bacc(
        "TRN2",
        target_bir_lowering=False,
        debug=False,
        enable_asserts=False,
        num_devices=NCORES,
    )

    LD_MAX = 320
    rt_d = nc.dram_tensor("rt", [P, NCH * M], bf16, kind="ExternalInput")
    rtab_d = nc.dram_tensor("rtab", [P, 2 * NCH * LD_MAX], bf16, kind="ExternalInput")
    bias_d = nc.dram_tensor("bias", [P, NCH * 2 * NMA], f32, kind="ExternalInput")
    mask_d = nc.dram_tensor("mask", [NS, M], f32, kind="ExternalInput")
    out_d = nc.dram_tensor("out", [NS, 1], f32, kind="ExternalOutput")

    with ExitStack() as ctx:
        tc = ctx.enter_context(tile.TileContext(nc))
        singles = ctx.enter_context(tc.tile_pool(name="singles", bufs=1))
        epool = ctx.enter_context(tc.tile_pool(name="e", bufs=4))
        e2pool = ctx.enter_context(tc.tile_pool(name="e2", bufs=4))
        pspool = ctx.enter_context(tc.tile_pool(name="ps", bufs=1, space="PSUM"))

        rt_sb = singles.tile([P, NCH, M], bf16)
        rtab_sb = singles.tile([P, 2, NCH, LD_MAX], bf16)
        bias_sb = singles.tile([P, NCH, 2 * NMA], f32)
        mask0 = singles.tile([NR, M], f32)
        mask1 = singles.tile([NR, M], f32)

        rt_view = rt_d.ap().rearrange("p (c t) -> p c t", c=NCH)
        rtab_view = rtab_d.ap().rearrange("p (s c t) -> p s c t", s=2, c=NCH)

        # sync (hw) queue carries the critical-path inputs in need-order:
        # D-groups (k desc) touch rtab cols desc and rt [176:512] first
        nc.sync.dma_start(
            out=rtab_sb[:, :, :, 112:LD_MAX], in_=rtab_view[:, :, :, 112:LD_MAX]
        )
        nc.sync.dma_start(out=rt_sb[:, :, 176:M], in_=rt_view[:, :, 176:M])
        nc.sync.dma_start(out=rtab_sb[:, :, :, 0:112], in_=rtab_view[:, :, :, 0:112])
        nc.sync.dma_start(out=rt_sb[:, :, 0:176], in_=rt_view[:, :, 0:176])
        # small/late inputs on the gpsimd queue
        nc.gpsimd.dma_start(
            out=bias_sb, in_=bias_d.ap().rearrange("p (c s) -> p c s", c=NCH)
        )
        nc.gpsimd.dma_start(out=mask0, in_=mask_d.ap()[0:NR, :])
        nc.gpsimd.dma_start(out=mask1, in_=mask_d.ap()[NR:NS, :])

        # one-hot lhsT bank: col r of view [:, :, 32-r : 64-r] is 1
        oh8 = singles.tile([P, 2, 2 * NR], fp8)
        nc.vector.memset(oh8, 0.0)
        nc.vector.memset(oh8[:, :, NR : NR + 1], 1.0)

        ps0 = pspool.tile([NR, M], f32)
        ps1 = pspool.tile([NR, M], f32)
        nc.vector.memset(ps0, 0.0)
        nc.vector.memset(ps1, 0.0)
        pss = [ps0, ps1]

        # HAM keep-warm: PE re-throttles unless its activity windows stay
        # full. Small filler matmuls (never read) emitted after each group
        # bridge the PE idle gaps while the next group's square lands.
        warmps = pspool.tile([NR, M], f32)
        dum = singles.tile([P, 2, 256], fp8)
        nc.vector.memset(dum, 0.0)

        def _pe_fill(n, cols=128):
            for _ in range(n):
                nc.tensor.matmul(
                    warmps[:, 0:cols],
                    oh8[:, :, NR : NR + NR],
                    dum[:, :, 0:cols],
                    start=True,
                    stop=True,
                    skip_group_check=True,
                    perf_mode=mybir.MatmulPerfMode.DoubleRow,
                )

        _pe_fill(24, 256)

        for step_i, (kind, idx) in enumerate(_emit_order()):
            ps = pss[step_i % 2]
            r_base = 2 * (step_i // 2)
            e_t = epool.tile([P, 2, NCH, M], bf16, tag="e")

            if kind == "A":
                m = idx
                L = M - 16 * m
                for sl in range(2):
                    s = 2 * m + sl
                    for c in range(NCH):
                        nc.vector.tensor_scalar(
                            out=e_t[:, sl, c, 0:L],
                            in0=rt_sb[:, c, 16 * m : M],
                            scalar1=bias_sb[:, c, s : s + 1],
                            scalar2=0.0,
                            op0=AOT.subtract,
                            op1=AOT.max,
                        )
            else:
                k = idx
                L = LD_MAX - 16 * k
                in0s = rt_sb[:, :, TMIN : TMIN + L]
                in0 = bass.AP(
                    tensor=in0s.tensor,
                    offset=in0s.offset,
                    ap=[in0s.ap[0], [0, 2], in0s.ap[1], in0s.ap[2]],
                )
                nc.vector.tensor_sub(
                    e_t[:, :, :, 0:L],
                    in0,
                    rtab_sb[:, :, :, 16 * k : 16 * k + L],
                )
                nc.vector.tensor_scalar(
                    out=e_t[:, :, :, 0:L],
                    in0=e_t[:, :, :, 0:L],
                    scalar1=0.0,
                    scalar2=None,
                    op0=AOT.max,
                )

            e2 = e2pool.tile([P, 2, NCH, M], fp8, tag="e2")
            if (kind, idx) in SQ_DVE_STEPS:
                nc.vector.tensor_mul(
                    e2[:, :, :, 0:L], e_t[:, :, :, 0:L], e_t[:, :, :, 0:L]
                )
            else:
                nc.scalar.activation(
                    out=e2[:, :, :, 0:L], in_=e_t[:, :, :, 0:L], func=AFT.Square
                )
            for sl in range(2):
                r = r_base + sl
                for c2 in range(NCH // 2):
                    nc.tensor.matmul(
                        ps[:, 0:L],
                        oh8[:, :, NR - r : 2 * NR - r],
                        e2[:, sl, 2 * c2 : 2 * c2 + 2, 0:L],
                        start=False,
                        stop=False,
                        skip_group_check=True,
                        perf_mode=mybir.MatmulPerfMode.DoubleRow,
                    )
            _pe_fill(3)

        for par, (ps, msk) in enumerate(((ps0, mask0), (ps1, mask1))):
            t_m = singles.tile([NR, M], f32, name=f"t{par}")
            nc.vector.tensor_mul(t_m, ps, msk)
            sq = singles.tile([NR, M], bf16, name=f"sq{par}")
            res = singles.tile([NR, 1], f32, name=f"res{par}")
            nc.scalar.activation(out=sq, in_=t_m, func=AFT.Sqrt, accum_out=res)
            nc.sync.dma_start(out=out_d.ap()[par * NR : (par + 1) * NR, :], in_=res)

    nc.compile()
    _PROG["nc"] = nc
    return nc


def _slot_infos():
    """Device-order slot list: (parity, row, kind, idx, sl)."""
    infos = []
    for step_i, (kind, idx) in enumerate(_emit_order()):
        for sl in range(2):
            infos.append((step_i % 2, 2 * (step_i // 2) + sl, kind, idx, sl))
    return infos


def _in_maps(repr_np, GT_np):
    import ml_dtypes

    LD_MAX = 320
    r = np.asarray(repr_np, dtype=np.float32)[np.asarray(GT_np).astype(np.int64)]
    rT = np.ascontiguousarray(r.T)  # [N, M] f32
    rT_bf = rT.astype(ml_dtypes.bfloat16)

    rt = np.ascontiguousarray(
        np.transpose(rT_bf.reshape(NCH, P, M), (1, 0, 2))
    ).reshape(P, -1)
    rT_f32_p = np.transpose(rT.reshape(NCH, P, M), (1, 0, 2))  # [P, NCH, M] f32

    infos = _slot_infos()
    maps = []
    for c in range(NCORES):
        js_a = []
        for m in range(NMA):
            js_a += [16 * m + c, 16 * m + 15 - c]
        bias = np.ascontiguousarray(rT_f32_p[:, :, js_a].astype(np.float32)).reshape(
            P, -1
        )

        rtab = np.full((2, N, LD_MAX), HUGE, dtype=np.float32)
        for sl, delta in enumerate((c + 1, 16 - c)):
            shift = TMIN + delta
            take = min(M - shift, LD_MAX)
            rtab[sl, :, :take] = rT[:, shift : shift + take]
        rtab_bf = rtab.astype(ml_dtypes.bfloat16)
        rtab_p = np.transpose(rtab_bf.reshape(2, NCH, P, LD_MAX), (2, 0, 1, 3))
        rtab_p = np.ascontiguousarray(rtab_p).reshape(P, -1)

        # mask rows: [parity*NR + row, u]
        mask = np.zeros((NS, M), dtype=np.float32)
        for par, row, kind, idx, sl in infos:
            mrow = par * NR + row
            if kind == "A":
                m = idx
                L = M - 16 * m
                j = 16 * m + c if sl == 0 else 16 * m + 15 - c
                mask[mrow, j - 16 * m + 1 : L] = 1.0
            else:
                k = idx
                L = LD_MAX - 16 * k
                mask[mrow, 0:L] = 1.0

        maps.append({"rt": rt, "rtab": rtab_p, "bias": bias, "mask": mask})
    return maps


def run_device(repr_np, GT_np, trace=False, trace_cores=None):
    """Run the bass kernel on 8 cores; returns (total, BassKernelResults)."""
    from concourse.bass_utils import run_bass_kernel_spmd

    nc = _build_program()
    maps = _in_maps(repr_np, GT_np)
    res = run_bass_kernel_spmd(
        nc,
        maps,
        core_ids=list(range(NCORES)),
        trace=trace,
        trace_cores=trace_cores,
    )
    total = 0.0
    for core_out in res.results:
        total += float(core_out["out"].astype(np.float64).sum())
    return np.float32(total), res


def kernel(repr, GT):
    total, _ = run_device(repr, GT, trace=False)
    return total
